# revision 63
# baseline (speedup 1.0000x reference)
"""Multi-head attention (per-head projections + relative position bias) on 8
Trainium2 NeuronCores.

Sharding: core c -> batch c//4, heads 4*(c%4) .. 4*(c%4)+4 (tensor parallel
over heads within a batch). Each core computes its 4 heads end-to-end plus the
partial output projection for those heads; the host sums the 4 partials per
batch and adds bfc.

v2 design (vs the 315us v1; measures ~262-275us): the whole kernel is one
stream of 128 "units" (head-pair pr, q-quarter qq of 512, t-tile tt of 128):
- Scores for BOTH heads of a pair land in ONE [128t, 1024] PSUM tile
  (hh0 cols 0:512 | hh1 cols 512:1024) via two CONCURRENT row-tiled
  K=64 matmuls (tile_position (0,0)/(64,0)), so QK runs at full PE width.
- One N=1024 Exp per unit: the scalar engine does exps (plus one ln/exp
  reciprocal pass per q-quarter, deferred a few units so it never
  chain-stalls the exp stream).
- AV matmuls are emitted SKEW units late so the tensor engine is never
  parked behind the exp->mult chain of a recent unit (the multiplies
  alternate DVE / gpsimd, and gpsimd ones are ~2.2us).
- Projections (Q/K per 512-col chunk, V per t-tile JIT before its first
  AV) and the output FC are woven into tensor slack at scheduled units,
  sharing a 2-bank PSUM rotation; there is no serial phase A.
- DMA bandwidth is fair-shared across everything in flight (~180GB/s/core
  during the 8-core ramp), so the ramp keeps only first-unit-critical
  bytes free-running; all other fetches are scheduled behind the first
  gpsimd multiply, a natural fence.
- Softmax denominators via a 65th ones-column on V; all PSUM drains on
  DVE; eb tiles stream on the sync queue, x/out on the gpsimd queue.
PSUM: 4 banks score slots + 2 banks AV accumulators + 2 banks misc rotation.
"""

import sys

sys.path.insert(0, "/opt/trn_rl_repo")

import numpy as np

import concourse.bass as bass
import concourse.tile as tile_mod
from concourse import mybir

# ---------------------------------------------------------------------------
# This walrus build accepts only one sem-wait per CTRL/Drain instruction, so
# split the TileContext tail drain's waits onto individual single-wait nops.
# ---------------------------------------------------------------------------


def _patched_drain_and_barrier(self, tick_clock, wait_clock):
    nc = self.nc
    drain_inst = nc.sync.drain()
    wait_clock.add_sem_waits(
        drain_inst.ins, tile_mod.ScopedClock({None: tick_clock.global_clock})
    )
    si = drain_inst.ins.sync_info
    if si is not None and si.on_wait is not None and len(si.on_wait) > 1:
        waits = list(si.on_wait)
        si.on_wait = [waits[0]]
        for w in waits[1:]:
            n = nc.sync.nop()
            n.ins.sync_info = mybir.SyncInfo(on_wait=[w], on_update=[])

    nc.all_engine_barrier()
    assert self.sems is not None
    popped = nc._tile_sem_poison_stack.pop()
    assert popped is self._sem_poison
    nc.clear_and_free_semaphores(list(self.sems.allocated().values()))
    nc.all_engine_barrier()


tile_mod.TileContext._drain_and_barrier = _patched_drain_and_barrier

_split_ctr = [0]


def _split_multi_waits(nc):
    """Walrus here accepts a single sem-wait per instruction; hoist extra waits
    onto single-wait nops inserted just before, on the same engine."""
    for f in nc.m.functions:
        for bb in f.blocks:
            insts = bb.instructions
            out = []
            for inst in insts:
                si = inst.sync_info
                if si is not None and si.on_wait is not None and len(si.on_wait) > 1:
                    waits = list(si.on_wait)
                    for w in waits[:-1]:
                        _split_ctr[0] += 1
                        n = mybir.InstNoOp(name=f"splitw-{_split_ctr[0]}", ins=[], outs=[])
                        n.engine = inst.engine
                        n.sync_info = mybir.SyncInfo(on_wait=[w], on_update=[])
                        out.append(n)
                    inst.sync_info = mybir.SyncInfo(
                        on_wait=[waits[-1]], on_update=list(si.on_update or [])
                    )
                out.append(inst)
            if len(out) != len(insts):
                bb.instructions[:] = out


B, S, D, H, DH = 2, 2048, 1024, 16, 64
NCORES = 8
HPC = 4  # heads per core
P = 128
F16 = mybir.dt.float16
F32 = mybir.dt.float32
AF = mybir.ActivationFunctionType
OP = mybir.AluOpType

NU = 128  # units: (qq 4) x (pr 2) x (tt 16)
GPSIMD_MULT_EVERY = 3  # every Nth es*eb multiply runs on gpsimd (0 = never)
SKEW = 3  # AV for unit u is emitted on the tensor queue during unit u+SKEW

_cached = {}


def _build_program(split_waits=True):
    nc = bass.Bass("TRN2", target_bir_lowering=False, debug=False)

    # x tensors pre-arranged on host: [p, scol, dp, s512]
    d_xq = nc.dram_tensor("xq", [P, 4, 8, 512], F16, kind="ExternalInput").ap()
    d_xk = nc.dram_tensor("xk", [P, 4, 8, 512], F16, kind="ExternalInput").ap()
    d_xv = nc.dram_tensor("xv", [P, 4, 8, 512], F16, kind="ExternalInput").ap()
    # eb[pr, qq, tt, t(128), (hh q512)] = exp(bias/8) pre-tiled
    d_eb = nc.dram_tensor("eb", [2, 4, 16, P, 1024], F16, kind="ExternalInput").ap()
    # per-pr halves as separate tensors so the pr1 halves can be DMA-deferred
    d_wq = [
        nc.dram_tensor(f"wq{pr}", [P, 8, P], F16, kind="ExternalInput").ap()
        for pr in range(2)
    ]
    d_wk = [
        nc.dram_tensor(f"wk{pr}", [P, 8, P], F16, kind="ExternalInput").ap()
        for pr in range(2)
    ]
    d_bq = nc.dram_tensor("bq", [2, P, 1], F32, kind="ExternalInput").ap()
    d_bk = nc.dram_tensor("bk", [2, P, 1], F32, kind="ExternalInput").ap()
    d_wv = nc.dram_tensor("wv", [P, 8, HPC * 65], F16, kind="ExternalInput").ap()
    d_bv = nc.dram_tensor("bv", [1, HPC * 65], F16, kind="ExternalInput").ap()
    # wfc[j(128 = pair heads stacked), pair, e(1024)]
    d_wfc = nc.dram_tensor("wfc", [P, 2, D], F16, kind="ExternalInput").ap()
    d_out = nc.dram_tensor("out", [S, D], F16, kind="ExternalOutput").ap()

    with tile_mod.TileContext(nc) as tc:
        with tc.tile_pool(name="persist", bufs=1) as persist, \
             tc.tile_pool(name="ebpool", bufs=6) as ebpool, \
             tc.tile_pool(name="espool", bufs=5) as espool, \
             tc.tile_pool(name="esbpool", bufs=5) as esbpool, \
             tc.tile_pool(name="xkp", bufs=3) as xkp, \
             tc.tile_pool(name="xqp", bufs=2) as xqp, \
             tc.tile_pool(name="xvp", bufs=3) as xvp, \
             tc.tile_pool(name="otp", bufs=6) as otp, \
             tc.tile_pool(name="rsp", bufs=2) as rsp, \
             tc.tile_pool(name="rbp", bufs=4) as rbp, \
             tc.tile_pool(name="fop", bufs=4) as fop, \
             tc.tile_pool(name="ps_sc", bufs=2, space="PSUM") as ps_sc, \
             tc.tile_pool(name="ps_po", bufs=2, space="PSUM") as ps_po, \
             tc.tile_pool(name="ps_mi", bufs=2, space="PSUM") as ps_mi:

            qT = persist.tile([P, 2, S], F16, tag="qT")  # [j(2 heads), pr, s]
            kT = persist.tile([P, 2, S], F16, tag="kT")
            vv = persist.tile([P, 16, HPC * 65], F16, tag="vv")  # [t_in, tt, h*65+j]
            onorm2 = persist.tile([P, 2, S], F16, tag="onorm2")
            wfc_sb = persist.tile([P, 2, D], F16, tag="wfc")
            wq_sb = persist.tile([P, 2, 8, P], F16, tag="wq")
            wk_sb = persist.tile([P, 2, 8, P], F16, tag="wk")
            wv_sb = persist.tile([P, 8, HPC * 65], F16, tag="wv")
            bq_sb = persist.tile([P, 2], F32, tag="bq")
            bk_sb = persist.tile([P, 2], F32, tag="bk")
            ones_sb = persist.tile([1, P], F16, tag="ones")
            ones4 = persist.tile([97, 64], F16, tag="ones4")
            bv_sb = persist.tile([1, HPC * 65], F16, tag="bv")
            bv_bc = persist.tile([P, HPC * 65], F16, tag="bv_bc")

            nc.vector.memset(ones_sb[:], 1.0)
            nc.vector.memset(ones4[:], 1.0)
            # Ramp-critical weights only: DMA bandwidth is fair-shared across
            # everything in flight, so anything not needed for the first few
            # units is deferred behind the first gpsimd multiply (a natural
            # fence at ~unit-2 time). wq/wk split per-pr for the same reason.
            nc.gpsimd.dma_start(wv_sb[:], d_wv[:])
            nc.gpsimd.dma_start(bv_sb[:], d_bv[:])
            nc.gpsimd.dma_start(wq_sb[:, 0], d_wq[0][:])
            nc.gpsimd.dma_start(wk_sb[:, 0], d_wk[0][:])
            nc.gpsimd.dma_start(bq_sb[:], d_bq.rearrange("pr p one -> p (pr one)"))
            nc.gpsimd.dma_start(bk_sb[:], d_bk.rearrange("pr p one -> p (pr one)"))

            # ------------- x chunk streaming -------------
            xk_tiles = {}
            xq_tiles = {}
            xv_tiles = {}

            def fetch_x(which, scol, queue="gpsimd"):
                pool, d_x, tiles = {
                    "k": (xkp, d_xk, xk_tiles),
                    "q": (xqp, d_xq, xq_tiles),
                    "v": (xvp, d_xv, xv_tiles),
                }[which]
                t = pool.tile([P, 8, 512], F16, tag=f"x{which}", name=f"x{which}{scol}")
                getattr(nc, queue).dma_start(t[:], d_x[:, scol])
                tiles[scol] = t

            # ------------- weave items (ride the ps_mi PSUM rotation) -------------
            def qk_proj(which, scol, pr, pool=None, tag="misc"):
                w_sb, x_tiles, b_sb, dst = {
                    "q": (wq_sb, xq_tiles, bq_sb, qT),
                    "k": (wk_sb, xk_tiles, bk_sb, kT),
                }[which]
                x_sb = x_tiles[scol]
                ps = (pool or ps_mi).tile(
                    [P, 512], F32, tag=tag, name=f"pj{which}{scol}{pr}"
                )
                for dp in range(8):
                    nc.tensor.matmul(
                        ps[:],
                        lhsT=w_sb[:, pr, dp, :],
                        rhs=x_sb[:, dp, :],
                        start=(dp == 0),
                        stop=(dp == 7),
                    )
                nc.vector.tensor_scalar(
                    dst[:, pr, scol * 512 : (scol + 1) * 512],
                    ps[:],
                    b_sb[:, pr : pr + 1],
                    None,
                    OP.add,
                )

            def v_proj(tt, pool=None, tag="misc"):
                x_sb = xv_tiles[tt // 4]
                c0 = (tt % 4) * P
                psv = (pool or ps_mi).tile([P, 512], F32, tag=tag, name=f"psv{tt}")
                for dp in range(8):
                    nc.tensor.matmul(
                        psv[:, 0 : HPC * 65],
                        lhsT=x_sb[:, dp, c0 : c0 + P],
                        rhs=wv_sb[:, dp, :],
                        start=(dp == 0),
                        stop=(dp == 7),
                    )
                # bias + the rowsum ones-column ride the drain as a broadcast
                # add (bv_bc is built once in the ramp)
                nc.vector.tensor_tensor(
                    vv[:, tt, :], psv[:, 0 : HPC * 65], bv_bc[:], OP.add
                )

            def fc_chunk(qt, ec):
                psf = ps_mi.tile([P, 512], F32, tag="misc", name=f"psf{qt}{ec}")
                for p2 in range(2):
                    nc.tensor.matmul(
                        psf[:],
                        lhsT=onorm2[:, p2, qt * P : (qt + 1) * P],
                        rhs=wfc_sb[:, p2, ec * 512 : (ec + 1) * 512],
                        start=(p2 == 0),
                        stop=(p2 == 1),
                    )
                fo = fop.tile([P, 512], F16, tag="fo", name=f"fo{qt}{ec}")
                nc.vector.tensor_copy(fo[:], psf[:])
                nc.sync.dma_start(
                    d_out[qt * P : (qt + 1) * P, ec * 512 : (ec + 1) * 512], fo[:]
                )

            # ------------- weave schedule: unit -> list of closures -------------
            weave = {u: [] for u in range(NU + 1)}

            def sched(u, fn):
                weave[min(u, NU)].append(fn)

            # V projections tt 4..15 spread one per unit (tt 0..3 in ramp);
            # vv[tt] is consumed by AV(g0, tt) at unit tt+SKEW.
            # All deferred DMA posts ride the gpsimd queue at units >= 2, i.e.
            # behind MULT(2) -- they cannot steal ramp bandwidth.
            sched(2, lambda: fetch_x("k", 1))
            sched(2, lambda: fetch_x("v", 1))
            sched(2, lambda: nc.gpsimd.dma_start(wq_sb[:, 1], d_wq[1][:]))
            sched(2, lambda: nc.gpsimd.dma_start(wk_sb[:, 1], d_wk[1][:]))
            for tt in range(4, 16):
                sched(tt, (lambda t=tt: v_proj(t)))
            # kT: (s0,pr0) in ramp; pr0 scols needed at unit 4*s, pr1 at 16+4*s
            sched(3, lambda: qk_proj("k", 1, 0))
            sched(3, lambda: fetch_x("k", 2))
            sched(3, lambda: fetch_x("v", 2))
            sched(5, lambda: qk_proj("k", 0, 1))
            sched(5, lambda: fetch_x("k", 3))
            sched(5, lambda: fetch_x("v", 3))
            sched(7, lambda: qk_proj("k", 2, 0))
            sched(8, lambda: qk_proj("q", 0, 1))
            sched(9, lambda: nc.gpsimd.dma_start(wfc_sb[:], d_wfc[:]))
            sched(10, lambda: qk_proj("k", 3, 0))
            sched(11, lambda: qk_proj("k", 1, 1))
            sched(13, lambda: qk_proj("k", 2, 1))
            sched(15, lambda: qk_proj("k", 3, 1))
            # qT: scol i needed at unit 32*i (pr0) / 32*i+16 (pr1)
            for i in range(1, 4):
                sched(32 * (i - 1) + 20, (lambda s=i: fetch_x("q", s)))
                sched(32 * (i - 1) + 24, (lambda s=i: qk_proj("q", s, 0)))
                sched(32 * (i - 1) + 27, (lambda s=i: qk_proj("q", s, 1)))
            # one ln/exp per qq a few units after its last AV drain, then the
            # per-pair broadcasts, then FC during qq+1 (qq3 clamps to flush)
            for qq in range(4):
                sched(32 * (qq + 1) + 5, (lambda q=qq: norm_ln(q)))
                sched(32 * (qq + 1) + 6, (lambda q=qq: norm_half(q, 0)))
                sched(32 * (qq + 1) + 7, (lambda q=qq: norm_half(q, 1)))
                for j, (qt, ec) in enumerate(
                    [(qt, ec) for qt in range(4 * qq, 4 * qq + 4) for ec in range(2)]
                ):
                    sched(32 * (qq + 1) + 9 + 3 * j, (lambda a=qt, b=ec: fc_chunk(a, b)))

            # ------------- ramp -------------
            # keep ramp DMA in-flight minimal so bandwidth fair-sharing does
            # not delay the critical chunks; kT/qT proj lead the tensor queue
            # (they gate the first exp), V0..V3 follow once xv0 lands
            fetch_x("k", 0, queue="sync")
            fetch_x("q", 0, queue="sync")
            fetch_x("v", 0)
            # bv broadcast [P, 260] built once: one K=1 matmul + drain
            ps_bv = ps_mi.tile([P, 512], F32, tag="misc", name="ps_bv")
            nc.tensor.matmul(
                ps_bv[:, 0 : HPC * 65],
                lhsT=ones_sb[:, 0:P],
                rhs=bv_sb[:],
                start=True,
                stop=True,
            )
            nc.vector.tensor_copy(bv_bc[:], ps_bv[:, 0 : HPC * 65])
            v_proj(0)
            v_proj(1)
            v_proj(2)
            v_proj(3)
            qk_proj("k", 0, 0)
            qk_proj("q", 0, 0)
            # fence eb slots 2-5: dummy tiles written with a dep on the qT
            # drain, so only eb 0-1 (0.5MB) free-fire into the ramp's DMA
            # flood; the rest start once the ramp-critical chunks are done
            for i in range(4):
                f = ebpool.tile([P, 1024], F16, tag="eb", name=f"ebfence{i}")
                nc.vector.tensor_copy(f[0:1, 0:4], qT[0:1, 0, 0:4])

            # ------------- main unit loop -------------
            state = {}

            def emit_av(u):
                g, tt = u // 16, u % 16
                qq, pr = g // 2, g % 2
                if tt == 0:
                    state["po"] = [
                        ps_po.tile([65, 512], F32, tag="po", name=f"po{g}{hh}")
                        for hh in range(2)
                    ]
                po = state["po"]
                esb = state.pop(("esb", u))
                for hh in range(2):
                    h = 2 * pr + hh
                    nc.tensor.matmul(
                        po[hh][:],
                        lhsT=vv[:, tt, h * 65 : (h + 1) * 65],
                        rhs=esb[:, hh * 512 : (hh + 1) * 512],
                        start=(tt == 0),
                        stop=(tt == 15),
                    )
                if tt == 15:
                    # drain this pair's AV accumulators + rowsum rows; the
                    # normalization itself runs a few units later (norm_half)
                    # so its ACT ln/exp never chain-stalls the exp stream
                    if pr == 0:
                        state[("rs", qq)] = rsp.tile(
                            [97, 512], F32, tag="rs", name=f"rs{qq}"
                        )
                        state[("lnr", qq)] = rsp.tile(
                            [97, 512], F32, tag="lnr", name=f"ln{qq}"
                        )
                        state[("rec", qq)] = rsp.tile(
                            [97, 512], F16, tag="rec", name=f"rc{qq}"
                        )
                    rs = state[("rs", qq)]
                    oTs = {}
                    for hh in range(2):
                        k4 = 2 * pr + hh
                        oT = otp.tile([64, 512], F16, tag="oT", name=f"oT{g}{hh}")
                        nc.vector.tensor_copy(oT[:], po[hh][0:64, :])
                        nc.vector.tensor_copy(
                            rs[32 * k4 : 32 * k4 + 1, :], po[hh][64:65, :]
                        )
                        oTs[hh] = oT
                    state[("oT", g)] = oTs

            def norm_ln(qq):
                # reciprocal as exp(-ln) on ACT (ln/exp share one table set
                # with the softmax exp); one pass covers all 4 rowsum rows
                rs = state[("rs", qq)]
                lnr = state[("lnr", qq)]
                rec16 = state[("rec", qq)]
                nc.scalar.activation(lnr[:], rs[:], AF.Ln)
                nc.scalar.activation(rec16[:], lnr[:], AF.Exp, scale=-1.0)

            def norm_half(qq, pr):
                # broadcast 1/rowsum along partitions via ones outer products
                # into the misc PSUM rotation, then scale this pair's output
                rec16 = state[("rec", qq)]
                oTs = state.pop(("oT", 2 * qq + pr))
                for hh in range(2):
                    k4 = 2 * pr + hh
                    rb = ps_mi.tile([P, 512], F32, tag="misc", name=f"rb{qq}{k4}")
                    nc.tensor.matmul(
                        rb[0:64, :],
                        lhsT=ones4[32 * k4 : 32 * k4 + 1, :],
                        rhs=rec16[32 * k4 : 32 * k4 + 1, :],
                        start=True,
                        stop=True,
                        tile_position=(32 * k4, 0),
                    )
                    nc.vector.tensor_tensor(
                        onorm2[
                            hh * 64 : (hh + 1) * 64,
                            pr,
                            qq * 512 : (qq + 1) * 512,
                        ],
                        oTs[hh][:],
                        rb[0:64, :],
                        OP.mult,
                    )

            for u in range(NU):
                g, tt = u // 16, u % 16
                qq, pr = g // 2, g % 2
                # eb prefetch (sync queue; pool depth gives lookahead)
                ebt = ebpool.tile([P, 1024], F16, tag="eb", name=f"eb{u}")
                nc.sync.dma_start(ebt[:], d_eb[pr, qq, tt])
                # scores: both heads concurrently via PE row tiles
                ps = ps_sc.tile([P, 1024], F32, tag="ps", name=f"ps{u}")
                for hh in range(2):
                    nc.tensor.matmul(
                        ps[:, hh * 512 : (hh + 1) * 512],
                        lhsT=kT[hh * 64 : (hh + 1) * 64, pr, tt * P : (tt + 1) * P],
                        rhs=qT[hh * 64 : (hh + 1) * 64, pr, qq * 512 : (qq + 1) * 512],
                        start=True,
                        stop=True,
                        tile_position=(hh * 64, 0),
                    )
                es = espool.tile([P, 1024], F16, tag="es", name=f"es{u}")
                nc.scalar.activation(es[:], ps[:], AF.Exp, scale=0.125)
                esb = esbpool.tile([P, 1024], F16, tag="esb", name=f"esb{u}")
                eng = (
                    nc.gpsimd
                    if GPSIMD_MULT_EVERY
                    and (u % GPSIMD_MULT_EVERY == GPSIMD_MULT_EVERY - 1)
                    and u < NU - 6  # keep the flush off the slow gpsimd path
                    else nc.vector
                )
                eng.tensor_tensor(esb[:], es[:], ebt[:], OP.mult)
                state[("esb", u)] = esb
                # AV lags by SKEW units so the PE never parks behind the
                # exp->mult chain of a recent unit
                if u >= SKEW:
                    emit_av(u - SKEW)
                for fn in weave[u]:
                    fn()

            for u in range(NU - SKEW, NU):
                emit_av(u)
            for fn in weave[NU]:
                fn()

    if split_waits:
        _split_multi_waits(nc)
    return nc


def _prep_eb_all(relative_position_bias):
    """exp(bias/8) for the full tensor, f16, once."""
    return np.exp(
        0.125 * np.asarray(relative_position_bias, dtype=np.float32)
    ).astype(np.float16)


def _prep_core_inputs(c, query, key, value, eb_all, Wq, bq, Wk, bk, Wv, bv, Wfc):
    b = c // (NCORES // B)
    h0 = HPC * (c % (NCORES // B))
    f16 = np.float16

    # X: [D, S] transposed input -> device layout [p(128), scol(4), dp(8), 512]
    def xprep(x):
        xt = np.asarray(x.T, dtype=f16)  # [D, S]
        return np.ascontiguousarray(
            xt.reshape(8, P, 4, 512).transpose(1, 2, 0, 3)
        )

    xq = xprep(query[b])
    xk = xprep(key[b])
    xv = xprep(value[b])

    # eb tiles: [pr, qq, tt, t(128), hh, q(512)]
    # eb_all[b, h] is [q, t]; device wants [t, q].
    y = eb_all[b, h0 : h0 + HPC]  # [4, q, t] f16
    yt = y.transpose(0, 2, 1)  # [4, t, q]
    eb = np.ascontiguousarray(
        yt.reshape(2, 2, 16, P, 4, 512).transpose(0, 4, 2, 3, 1, 5)
    ).reshape(2, 4, 16, P, 1024)  # [pr, qq, tt, t, (hh q)]

    # wq/wk: per-pair [D, 128] -> [p, pr, dp, j]
    wq = np.stack(
        [np.concatenate([Wq[h0 + 2 * g], Wq[h0 + 2 * g + 1]], axis=1) for g in range(2)]
    ).astype(f16)  # [2, D, 128]
    wq = np.ascontiguousarray(wq.reshape(2, 8, P, P).transpose(2, 0, 1, 3))
    wk = np.stack(
        [np.concatenate([Wk[h0 + 2 * g], Wk[h0 + 2 * g + 1]], axis=1) for g in range(2)]
    ).astype(f16)
    wk = np.ascontiguousarray(wk.reshape(2, 8, P, P).transpose(2, 0, 1, 3))
    bqc = np.stack(
        [np.concatenate([bq[h0 + 2 * g], bq[h0 + 2 * g + 1]])[:, None] for g in range(2)]
    ).astype(np.float32)
    bkc = np.stack(
        [np.concatenate([bk[h0 + 2 * g], bk[h0 + 2 * g + 1]])[:, None] for g in range(2)]
    ).astype(np.float32)

    wv = np.zeros((D, HPC * 65), dtype=f16)
    bv_aug = np.zeros((1, HPC * 65), dtype=f16)
    for i in range(HPC):
        wv[:, i * 65 : i * 65 + 64] = Wv[h0 + i]
        bv_aug[0, i * 65 : i * 65 + 64] = bv[h0 + i]
        bv_aug[0, i * 65 + 64] = 1.0
    wv = np.ascontiguousarray(wv.reshape(8, P, HPC * 65).transpose(1, 0, 2))

    # wfc: [j(128 = pair heads stacked), pair, e]
    wfc = np.stack(
        [Wfc[(h0 + 2 * p) * DH : (h0 + 2 * p + 2) * DH] for p in range(2)]
    ).astype(f16)  # [2, 128, D]
    wfc = np.ascontiguousarray(wfc.transpose(1, 0, 2))

    return {
        "xq": xq, "xk": xk, "xv": xv, "eb": eb,
        "wq0": np.ascontiguousarray(wq[:, 0]),
        "wq1": np.ascontiguousarray(wq[:, 1]),
        "wk0": np.ascontiguousarray(wk[:, 0]),
        "wk1": np.ascontiguousarray(wk[:, 1]),
        "bq": bqc, "bk": bkc,
        "wv": wv, "bv": bv_aug, "wfc": wfc,
    }


def _install_ntff_hook():
    """The container's antenv stub lacks axon_hooks; synthesize it so
    trace=True can capture NTFF profiles via libaxon_pjrt.so ctypes calls."""
    import contextlib
    import ctypes
    import types

    import antenv

    if hasattr(antenv, "axon_hooks"):
        return
    so_path = "/opt/axon/libaxon_pjrt.so"
    try:
        lib = ctypes.CDLL(so_path)
    except OSError:
        return
    if not hasattr(lib, "axon_start_nrt_profile"):
        return
    lib.axon_start_nrt_profile.argtypes = [ctypes.POINTER(ctypes.c_int64), ctypes.c_size_t]
    lib.axon_start_nrt_profile.restype = ctypes.c_int64
    lib.axon_stop_nrt_profile.argtypes = [ctypes.c_char_p]
    lib.axon_stop_nrt_profile.restype = ctypes.c_int64

    @contextlib.contextmanager
    def _hook(output_dir, device_ids):
        import jax

        jax.devices()
        if device_ids:
            ids = (ctypes.c_int64 * len(device_ids))(*device_ids)
            rc = lib.axon_start_nrt_profile(ids, len(device_ids))
        else:
            rc = lib.axon_start_nrt_profile(None, 0)
        if rc != 0:
            raise RuntimeError(f"axon_start_nrt_profile rc={rc}")
        try:
            yield
        finally:
            n = lib.axon_stop_nrt_profile(str(output_dir).encode())
            print(f"profile: {n} file(s) written to {output_dir}", file=sys.stderr)

    mod = types.ModuleType("antenv.axon_hooks")
    mod._hook = _hook
    mod.get_axon_ntff_profile_hook = lambda: _hook
    mod.set_axon_ntff_profile_hook = lambda h: setattr(mod, "_hook", h)
    sys.modules["antenv.axon_hooks"] = mod
    antenv.axon_hooks = mod


def kernel(_trace=False, **inputs):
    from concourse.bass_utils import run_bass_kernel_spmd

    if _trace:
        _install_ntff_hook()
    if "nc" not in _cached:
        _cached["nc"] = _build_program()
    nc = _cached["nc"]

    args = {k: np.asarray(v) for k, v in inputs.items()}
    eb_all = _prep_eb_all(args["relative_position_bias"])
    in_maps = [
        _prep_core_inputs(
            c,
            args["query"], args["key"], args["value"],
            eb_all,
            args["Wq"], args["bq"], args["Wk"], args["bk"],
            args["Wv"], args["bv"], args["Wfc"],
        )
        for c in range(NCORES)
    ]

    res = run_bass_kernel_spmd(nc, in_maps, core_ids=list(range(NCORES)), trace=_trace)
    _cached["last_result"] = res

    out = np.zeros((B, S, D), dtype=np.float32)
    cpb = NCORES // B
    for b in range(B):
        out[b] = sum(
            res.results[b * cpb + i]["out"].astype(np.float32) for i in range(cpb)
        )
        out[b] += args["bfc"].astype(np.float32)[None, :]
    return out


# revision 64
# speedup vs baseline: 1.0323x; 1.0323x over previous
"""Multi-head attention (per-head projections + relative position bias) on 8
Trainium2 NeuronCores.

Sharding: core c -> batch c//4, heads 4*(c%4) .. 4*(c%4)+4 (tensor parallel
over heads within a batch). Each core computes its 4 heads end-to-end plus the
partial output projection for those heads; the host sums the 4 partials per
batch and adds bfc.

v2 design (vs the 315us v1; measures ~262-275us): the whole kernel is one
stream of 128 "units" (head-pair pr, q-quarter qq of 512, t-tile tt of 128):
- Scores for BOTH heads of a pair land in ONE [128t, 1024] PSUM tile
  (hh0 cols 0:512 | hh1 cols 512:1024) via two CONCURRENT row-tiled
  K=64 matmuls (tile_position (0,0)/(64,0)), so QK runs at full PE width.
- One N=1024 Exp per unit: the scalar engine does exps (plus one ln/exp
  reciprocal pass per q-quarter, deferred a few units so it never
  chain-stalls the exp stream).
- AV matmuls are emitted SKEW units late so the tensor engine is never
  parked behind the exp->mult chain of a recent unit (the multiplies
  alternate DVE / gpsimd, and gpsimd ones are ~2.2us).
- Projections (Q/K per 512-col chunk, V per t-tile JIT before its first
  AV) and the output FC are woven into tensor slack at scheduled units,
  sharing a 2-bank PSUM rotation; there is no serial phase A.
- DMA bandwidth is fair-shared across everything in flight (~180GB/s/core
  during the 8-core ramp), so the ramp keeps only first-unit-critical
  bytes free-running; all other fetches are scheduled behind the first
  gpsimd multiply, a natural fence.
- Softmax denominators via a 65th ones-column on V; all PSUM drains on
  DVE; eb tiles stream on the sync queue, x/out on the gpsimd queue.
PSUM: 4 banks score slots + 2 banks AV accumulators + 2 banks misc rotation.
"""

import sys

sys.path.insert(0, "/opt/trn_rl_repo")

import numpy as np

import concourse.bass as bass
import concourse.tile as tile_mod
from concourse import mybir

# ---------------------------------------------------------------------------
# This walrus build accepts only one sem-wait per CTRL/Drain instruction, so
# split the TileContext tail drain's waits onto individual single-wait nops.
# ---------------------------------------------------------------------------


def _patched_drain_and_barrier(self, tick_clock, wait_clock):
    nc = self.nc
    drain_inst = nc.sync.drain()
    wait_clock.add_sem_waits(
        drain_inst.ins, tile_mod.ScopedClock({None: tick_clock.global_clock})
    )
    si = drain_inst.ins.sync_info
    if si is not None and si.on_wait is not None and len(si.on_wait) > 1:
        waits = list(si.on_wait)
        si.on_wait = [waits[0]]
        for w in waits[1:]:
            n = nc.sync.nop()
            n.ins.sync_info = mybir.SyncInfo(on_wait=[w], on_update=[])

    nc.all_engine_barrier()
    assert self.sems is not None
    popped = nc._tile_sem_poison_stack.pop()
    assert popped is self._sem_poison
    nc.clear_and_free_semaphores(list(self.sems.allocated().values()))
    nc.all_engine_barrier()


tile_mod.TileContext._drain_and_barrier = _patched_drain_and_barrier

_split_ctr = [0]


def _split_multi_waits(nc):
    """Walrus here accepts a single sem-wait per instruction; hoist extra waits
    onto single-wait nops inserted just before, on the same engine."""
    for f in nc.m.functions:
        for bb in f.blocks:
            insts = bb.instructions
            out = []
            for inst in insts:
                si = inst.sync_info
                if si is not None and si.on_wait is not None and len(si.on_wait) > 1:
                    waits = list(si.on_wait)
                    for w in waits[:-1]:
                        _split_ctr[0] += 1
                        n = mybir.InstNoOp(name=f"splitw-{_split_ctr[0]}", ins=[], outs=[])
                        n.engine = inst.engine
                        n.sync_info = mybir.SyncInfo(on_wait=[w], on_update=[])
                        out.append(n)
                    inst.sync_info = mybir.SyncInfo(
                        on_wait=[waits[-1]], on_update=list(si.on_update or [])
                    )
                out.append(inst)
            if len(out) != len(insts):
                bb.instructions[:] = out


B, S, D, H, DH = 2, 2048, 1024, 16, 64
NCORES = 8
HPC = 4  # heads per core
P = 128
F16 = mybir.dt.float16
F32 = mybir.dt.float32
AF = mybir.ActivationFunctionType
OP = mybir.AluOpType

NU = 128  # units: (qq 4) x (pr 2) x (tt 16)
GPSIMD_MULT_EVERY = 3  # every Nth es*eb multiply runs on gpsimd (0 = never)
SKEW = 3  # AV for unit u is emitted on the tensor queue during unit u+SKEW

_cached = {}


def _build_program(split_waits=True):
    nc = bass.Bass("TRN2", target_bir_lowering=False, debug=False)

    # x tensors pre-arranged on host: [p, scol, dp, s512]
    d_xq = nc.dram_tensor("xq", [P, 4, 8, 512], F16, kind="ExternalInput").ap()
    d_xk = nc.dram_tensor("xk", [P, 4, 8, 512], F16, kind="ExternalInput").ap()
    d_xv = nc.dram_tensor("xv", [P, 4, 8, 512], F16, kind="ExternalInput").ap()
    # eb[pr, qq, tt, t(128), (hh q512)] = exp(bias/8) pre-tiled
    d_eb = nc.dram_tensor("eb", [2, 4, 16, P, 1024], F16, kind="ExternalInput").ap()
    # per-pr halves as separate tensors so the pr1 halves can be DMA-deferred
    d_wq = [
        nc.dram_tensor(f"wq{pr}", [P, 8, P], F16, kind="ExternalInput").ap()
        for pr in range(2)
    ]
    d_wk = [
        nc.dram_tensor(f"wk{pr}", [P, 8, P], F16, kind="ExternalInput").ap()
        for pr in range(2)
    ]
    d_bq = nc.dram_tensor("bq", [2, P, 1], F32, kind="ExternalInput").ap()
    d_bk = nc.dram_tensor("bk", [2, P, 1], F32, kind="ExternalInput").ap()
    d_wv = nc.dram_tensor("wv", [P, 8, HPC * 65], F16, kind="ExternalInput").ap()
    d_bv = nc.dram_tensor("bv", [1, HPC * 65], F16, kind="ExternalInput").ap()
    # wfc[j(128 = pair heads stacked), pair, e(1024)]
    d_wfc = nc.dram_tensor("wfc", [P, 2, D], F16, kind="ExternalInput").ap()
    d_out = nc.dram_tensor("out", [S, D], F16, kind="ExternalOutput").ap()

    with tile_mod.TileContext(nc) as tc:
        with tc.tile_pool(name="persist", bufs=1) as persist, \
             tc.tile_pool(name="ebpool", bufs=6) as ebpool, \
             tc.tile_pool(name="espool", bufs=5) as espool, \
             tc.tile_pool(name="esbpool", bufs=5) as esbpool, \
             tc.tile_pool(name="xkp", bufs=3) as xkp, \
             tc.tile_pool(name="xqp", bufs=2) as xqp, \
             tc.tile_pool(name="xvp", bufs=3) as xvp, \
             tc.tile_pool(name="otp", bufs=6) as otp, \
             tc.tile_pool(name="rsp", bufs=2) as rsp, \
             tc.tile_pool(name="rbp", bufs=4) as rbp, \
             tc.tile_pool(name="fop", bufs=4) as fop, \
             tc.tile_pool(name="ps_sc", bufs=2, space="PSUM") as ps_sc, \
             tc.tile_pool(name="ps_po", bufs=2, space="PSUM") as ps_po, \
             tc.tile_pool(name="ps_mi", bufs=2, space="PSUM") as ps_mi:

            qT = persist.tile([P, 2, S], F16, tag="qT")  # [j(2 heads), pr, s]
            kT = persist.tile([P, 2, S], F16, tag="kT")
            vv = persist.tile([P, 16, HPC * 65], F16, tag="vv")  # [t_in, tt, h*65+j]
            onorm2 = persist.tile([P, 2, S], F16, tag="onorm2")
            wfc_sb = persist.tile([P, 2, D], F16, tag="wfc")
            wq_sb = persist.tile([P, 2, 8, P], F16, tag="wq")
            wk_sb = persist.tile([P, 2, 8, P], F16, tag="wk")
            wv_sb = persist.tile([P, 8, HPC * 65], F16, tag="wv")
            bq_sb = persist.tile([P, 2], F32, tag="bq")
            bk_sb = persist.tile([P, 2], F32, tag="bk")
            ones_sb = persist.tile([1, P], F16, tag="ones")
            ones4 = persist.tile([97, 64], F16, tag="ones4")
            bv_sb = persist.tile([1, HPC * 65], F16, tag="bv")
            bv_bc = persist.tile([P, HPC * 65], F16, tag="bv_bc")

            nc.vector.memset(ones_sb[:], 1.0)
            nc.vector.memset(ones4[:], 1.0)
            # Ramp-critical weights only: DMA bandwidth is fair-shared across
            # everything in flight, so anything not needed for the first few
            # units is deferred behind the first gpsimd multiply (a natural
            # fence at ~unit-2 time). wq/wk split per-pr for the same reason.
            nc.gpsimd.dma_start(wv_sb[:], d_wv[:])
            nc.gpsimd.dma_start(bv_sb[:], d_bv[:])
            nc.gpsimd.dma_start(wq_sb[:, 0], d_wq[0][:])
            nc.gpsimd.dma_start(wk_sb[:, 0], d_wk[0][:])
            nc.gpsimd.dma_start(bq_sb[:], d_bq.rearrange("pr p one -> p (pr one)"))
            nc.gpsimd.dma_start(bk_sb[:], d_bk.rearrange("pr p one -> p (pr one)"))

            # ------------- x chunk streaming -------------
            xk_tiles = {}
            xq_tiles = {}
            xv_tiles = {}

            def fetch_x(which, scol, queue="gpsimd"):
                pool, d_x, tiles = {
                    "k": (xkp, d_xk, xk_tiles),
                    "q": (xqp, d_xq, xq_tiles),
                    "v": (xvp, d_xv, xv_tiles),
                }[which]
                t = pool.tile([P, 8, 512], F16, tag=f"x{which}", name=f"x{which}{scol}")
                getattr(nc, queue).dma_start(t[:], d_x[:, scol])
                tiles[scol] = t

            # ------------- weave items (ride the ps_mi PSUM rotation) -------------
            def qk_proj(which, scol, pr, pool=None, tag="misc"):
                w_sb, x_tiles, b_sb, dst = {
                    "q": (wq_sb, xq_tiles, bq_sb, qT),
                    "k": (wk_sb, xk_tiles, bk_sb, kT),
                }[which]
                x_sb = x_tiles[scol]
                ps = (pool or ps_mi).tile(
                    [P, 512], F32, tag=tag, name=f"pj{which}{scol}{pr}"
                )
                for dp in range(8):
                    nc.tensor.matmul(
                        ps[:],
                        lhsT=w_sb[:, pr, dp, :],
                        rhs=x_sb[:, dp, :],
                        start=(dp == 0),
                        stop=(dp == 7),
                    )
                nc.vector.tensor_scalar(
                    dst[:, pr, scol * 512 : (scol + 1) * 512],
                    ps[:],
                    b_sb[:, pr : pr + 1],
                    None,
                    OP.add,
                )

            def v_proj(tt, pool=None, tag="misc"):
                x_sb = xv_tiles[tt // 4]
                c0 = (tt % 4) * P
                psv = (pool or ps_mi).tile([P, 512], F32, tag=tag, name=f"psv{tt}")
                for dp in range(8):
                    nc.tensor.matmul(
                        psv[:, 0 : HPC * 65],
                        lhsT=x_sb[:, dp, c0 : c0 + P],
                        rhs=wv_sb[:, dp, :],
                        start=(dp == 0),
                        stop=(dp == 7),
                    )
                # bias + the rowsum ones-column ride the drain as a broadcast
                # add (bv_bc is built once in the ramp)
                nc.vector.tensor_tensor(
                    vv[:, tt, :], psv[:, 0 : HPC * 65], bv_bc[:], OP.add
                )

            def fc_chunk(qt, ec):
                psf = ps_mi.tile([P, 512], F32, tag="misc", name=f"psf{qt}{ec}")
                for p2 in range(2):
                    nc.tensor.matmul(
                        psf[:],
                        lhsT=onorm2[:, p2, qt * P : (qt + 1) * P],
                        rhs=wfc_sb[:, p2, ec * 512 : (ec + 1) * 512],
                        start=(p2 == 0),
                        stop=(p2 == 1),
                    )
                fo = fop.tile([P, 512], F16, tag="fo", name=f"fo{qt}{ec}")
                nc.vector.tensor_copy(fo[:], psf[:])
                nc.sync.dma_start(
                    d_out[qt * P : (qt + 1) * P, ec * 512 : (ec + 1) * 512], fo[:]
                )

            # ------------- weave schedule: unit -> list of closures -------------
            weave = {u: [] for u in range(NU + 1)}

            def sched(u, fn):
                weave[min(u, NU)].append(fn)

            # V projections tt 4..15 spread one per unit (tt 0..3 in ramp);
            # vv[tt] is consumed by AV(g0, tt) at unit tt+SKEW.
            # All deferred DMA posts ride the gpsimd queue at units >= 2, i.e.
            # behind MULT(2) -- they cannot steal ramp bandwidth.
            sched(2, lambda: fetch_x("k", 1))
            sched(2, lambda: fetch_x("v", 1))
            sched(2, lambda: nc.gpsimd.dma_start(wq_sb[:, 1], d_wq[1][:]))
            sched(2, lambda: nc.gpsimd.dma_start(wk_sb[:, 1], d_wk[1][:]))
            for tt in range(4, 16):
                sched(tt, (lambda t=tt: v_proj(t)))
            # kT: (s0,pr0) in ramp; pr0 scols needed at unit 4*s, pr1 at 16+4*s
            sched(3, lambda: qk_proj("k", 1, 0))
            sched(3, lambda: fetch_x("k", 2))
            sched(3, lambda: fetch_x("v", 2))
            sched(5, lambda: qk_proj("k", 0, 1))
            sched(5, lambda: fetch_x("k", 3))
            sched(5, lambda: fetch_x("v", 3))
            sched(7, lambda: qk_proj("k", 2, 0))
            sched(8, lambda: qk_proj("q", 0, 1))
            sched(9, lambda: nc.gpsimd.dma_start(wfc_sb[:], d_wfc[:]))
            sched(10, lambda: qk_proj("k", 3, 0))
            sched(11, lambda: qk_proj("k", 1, 1))
            sched(13, lambda: qk_proj("k", 2, 1))
            sched(15, lambda: qk_proj("k", 3, 1))
            # qT: scol i needed at unit 32*i (pr0) / 32*i+16 (pr1)
            for i in range(1, 4):
                sched(32 * (i - 1) + 20, (lambda s=i: fetch_x("q", s)))
                sched(32 * (i - 1) + 24, (lambda s=i: qk_proj("q", s, 0)))
                sched(32 * (i - 1) + 27, (lambda s=i: qk_proj("q", s, 1)))
            # one ln/exp per qq a few units after its last AV drain, then the
            # per-pair broadcasts, then FC during qq+1 (qq3 clamps to flush)
            for qq in range(4):
                sched(32 * (qq + 1) + 5, (lambda q=qq: norm_ln(q)))
                sched(32 * (qq + 1) + 6, (lambda q=qq: norm_half(q, 0)))
                sched(32 * (qq + 1) + 7, (lambda q=qq: norm_half(q, 1)))
                for j, (qt, ec) in enumerate(
                    [(qt, ec) for qt in range(4 * qq, 4 * qq + 4) for ec in range(2)]
                ):
                    sched(32 * (qq + 1) + 9 + 3 * j, (lambda a=qt, b=ec: fc_chunk(a, b)))

            # ------------- ramp -------------
            # keep ramp DMA in-flight minimal so bandwidth fair-sharing does
            # not delay the critical chunks; kT/qT proj lead the tensor queue
            # (they gate the first exp), V0..V3 follow once xv0 lands
            fetch_x("k", 0, queue="sync")
            fetch_x("q", 0, queue="sync")
            fetch_x("v", 0)
            # bv broadcast [P, 260] built once: one K=1 matmul + drain
            ps_bv = ps_mi.tile([P, 512], F32, tag="misc", name="ps_bv")
            nc.tensor.matmul(
                ps_bv[:, 0 : HPC * 65],
                lhsT=ones_sb[:, 0:P],
                rhs=bv_sb[:],
                start=True,
                stop=True,
            )
            nc.vector.tensor_copy(bv_bc[:], ps_bv[:, 0 : HPC * 65])
            v_proj(0)
            v_proj(1)
            v_proj(2)
            v_proj(3)
            qk_proj("k", 0, 0)
            qk_proj("q", 0, 0)

            # ------------- main unit loop -------------
            state = {}

            def emit_av(u):
                g, tt = u // 16, u % 16
                qq, pr = g // 2, g % 2
                if tt == 0:
                    state["po"] = [
                        ps_po.tile([65, 512], F32, tag="po", name=f"po{g}{hh}")
                        for hh in range(2)
                    ]
                po = state["po"]
                esb = state.pop(("esb", u))
                for hh in range(2):
                    h = 2 * pr + hh
                    nc.tensor.matmul(
                        po[hh][:],
                        lhsT=vv[:, tt, h * 65 : (h + 1) * 65],
                        rhs=esb[:, hh * 512 : (hh + 1) * 512],
                        start=(tt == 0),
                        stop=(tt == 15),
                    )
                if tt == 15:
                    # drain this pair's AV accumulators + rowsum rows; the
                    # normalization itself runs a few units later (norm_half)
                    # so its ACT ln/exp never chain-stalls the exp stream
                    if pr == 0:
                        state[("rs", qq)] = rsp.tile(
                            [97, 512], F32, tag="rs", name=f"rs{qq}"
                        )
                        state[("lnr", qq)] = rsp.tile(
                            [97, 512], F32, tag="lnr", name=f"ln{qq}"
                        )
                        state[("rec", qq)] = rsp.tile(
                            [97, 512], F16, tag="rec", name=f"rc{qq}"
                        )
                    rs = state[("rs", qq)]
                    oTs = {}
                    for hh in range(2):
                        k4 = 2 * pr + hh
                        oT = otp.tile([64, 512], F16, tag="oT", name=f"oT{g}{hh}")
                        nc.vector.tensor_copy(oT[:], po[hh][0:64, :])
                        nc.vector.tensor_copy(
                            rs[32 * k4 : 32 * k4 + 1, :], po[hh][64:65, :]
                        )
                        oTs[hh] = oT
                    state[("oT", g)] = oTs

            def norm_ln(qq):
                # reciprocal as exp(-ln) on ACT (ln/exp share one table set
                # with the softmax exp); one pass covers all 4 rowsum rows
                rs = state[("rs", qq)]
                lnr = state[("lnr", qq)]
                rec16 = state[("rec", qq)]
                nc.scalar.activation(lnr[:], rs[:], AF.Ln)
                nc.scalar.activation(rec16[:], lnr[:], AF.Exp, scale=-1.0)

            def norm_half(qq, pr):
                # broadcast 1/rowsum along partitions via ones outer products
                # into the misc PSUM rotation, then scale this pair's output
                rec16 = state[("rec", qq)]
                oTs = state.pop(("oT", 2 * qq + pr))
                for hh in range(2):
                    k4 = 2 * pr + hh
                    rb = ps_mi.tile([P, 512], F32, tag="misc", name=f"rb{qq}{k4}")
                    nc.tensor.matmul(
                        rb[0:64, :],
                        lhsT=ones4[32 * k4 : 32 * k4 + 1, :],
                        rhs=rec16[32 * k4 : 32 * k4 + 1, :],
                        start=True,
                        stop=True,
                        tile_position=(32 * k4, 0),
                    )
                    nc.vector.tensor_tensor(
                        onorm2[
                            hh * 64 : (hh + 1) * 64,
                            pr,
                            qq * 512 : (qq + 1) * 512,
                        ],
                        oTs[hh][:],
                        rb[0:64, :],
                        OP.mult,
                    )

            for u in range(NU):
                g, tt = u // 16, u % 16
                qq, pr = g // 2, g % 2
                # eb prefetch (sync queue; pool depth gives lookahead)
                ebt = ebpool.tile([P, 1024], F16, tag="eb", name=f"eb{u}")
                nc.sync.dma_start(ebt[:], d_eb[pr, qq, tt])
                # scores: both heads concurrently via PE row tiles
                ps = ps_sc.tile([P, 1024], F32, tag="ps", name=f"ps{u}")
                for hh in range(2):
                    nc.tensor.matmul(
                        ps[:, hh * 512 : (hh + 1) * 512],
                        lhsT=kT[hh * 64 : (hh + 1) * 64, pr, tt * P : (tt + 1) * P],
                        rhs=qT[hh * 64 : (hh + 1) * 64, pr, qq * 512 : (qq + 1) * 512],
                        start=True,
                        stop=True,
                        tile_position=(hh * 64, 0),
                    )
                es = espool.tile([P, 1024], F16, tag="es", name=f"es{u}")
                nc.scalar.activation(es[:], ps[:], AF.Exp, scale=0.125)
                esb = esbpool.tile([P, 1024], F16, tag="esb", name=f"esb{u}")
                eng = (
                    nc.gpsimd
                    if GPSIMD_MULT_EVERY
                    and (u % GPSIMD_MULT_EVERY == GPSIMD_MULT_EVERY - 1)
                    and u < NU - 6  # keep the flush off the slow gpsimd path
                    else nc.vector
                )
                eng.tensor_tensor(esb[:], es[:], ebt[:], OP.mult)
                state[("esb", u)] = esb
                # AV lags by SKEW units so the PE never parks behind the
                # exp->mult chain of a recent unit
                if u >= SKEW:
                    emit_av(u - SKEW)
                for fn in weave[u]:
                    fn()

            for u in range(NU - SKEW, NU):
                emit_av(u)
            for fn in weave[NU]:
                fn()

    if split_waits:
        _split_multi_waits(nc)
    return nc


def _prep_eb_all(relative_position_bias):
    """exp(bias/8) for the full tensor, f16, once."""
    return np.exp(
        0.125 * np.asarray(relative_position_bias, dtype=np.float32)
    ).astype(np.float16)


def _prep_core_inputs(c, query, key, value, eb_all, Wq, bq, Wk, bk, Wv, bv, Wfc):
    b = c // (NCORES // B)
    h0 = HPC * (c % (NCORES // B))
    f16 = np.float16

    # X: [D, S] transposed input -> device layout [p(128), scol(4), dp(8), 512]
    def xprep(x):
        xt = np.asarray(x.T, dtype=f16)  # [D, S]
        return np.ascontiguousarray(
            xt.reshape(8, P, 4, 512).transpose(1, 2, 0, 3)
        )

    xq = xprep(query[b])
    xk = xprep(key[b])
    xv = xprep(value[b])

    # eb tiles: [pr, qq, tt, t(128), hh, q(512)]
    # eb_all[b, h] is [q, t]; device wants [t, q].
    y = eb_all[b, h0 : h0 + HPC]  # [4, q, t] f16
    yt = y.transpose(0, 2, 1)  # [4, t, q]
    eb = np.ascontiguousarray(
        yt.reshape(2, 2, 16, P, 4, 512).transpose(0, 4, 2, 3, 1, 5)
    ).reshape(2, 4, 16, P, 1024)  # [pr, qq, tt, t, (hh q)]

    # wq/wk: per-pair [D, 128] -> [p, pr, dp, j]
    wq = np.stack(
        [np.concatenate([Wq[h0 + 2 * g], Wq[h0 + 2 * g + 1]], axis=1) for g in range(2)]
    ).astype(f16)  # [2, D, 128]
    wq = np.ascontiguousarray(wq.reshape(2, 8, P, P).transpose(2, 0, 1, 3))
    wk = np.stack(
        [np.concatenate([Wk[h0 + 2 * g], Wk[h0 + 2 * g + 1]], axis=1) for g in range(2)]
    ).astype(f16)
    wk = np.ascontiguousarray(wk.reshape(2, 8, P, P).transpose(2, 0, 1, 3))
    bqc = np.stack(
        [np.concatenate([bq[h0 + 2 * g], bq[h0 + 2 * g + 1]])[:, None] for g in range(2)]
    ).astype(np.float32)
    bkc = np.stack(
        [np.concatenate([bk[h0 + 2 * g], bk[h0 + 2 * g + 1]])[:, None] for g in range(2)]
    ).astype(np.float32)

    wv = np.zeros((D, HPC * 65), dtype=f16)
    bv_aug = np.zeros((1, HPC * 65), dtype=f16)
    for i in range(HPC):
        wv[:, i * 65 : i * 65 + 64] = Wv[h0 + i]
        bv_aug[0, i * 65 : i * 65 + 64] = bv[h0 + i]
        bv_aug[0, i * 65 + 64] = 1.0
    wv = np.ascontiguousarray(wv.reshape(8, P, HPC * 65).transpose(1, 0, 2))

    # wfc: [j(128 = pair heads stacked), pair, e]
    wfc = np.stack(
        [Wfc[(h0 + 2 * p) * DH : (h0 + 2 * p + 2) * DH] for p in range(2)]
    ).astype(f16)  # [2, 128, D]
    wfc = np.ascontiguousarray(wfc.transpose(1, 0, 2))

    return {
        "xq": xq, "xk": xk, "xv": xv, "eb": eb,
        "wq0": np.ascontiguousarray(wq[:, 0]),
        "wq1": np.ascontiguousarray(wq[:, 1]),
        "wk0": np.ascontiguousarray(wk[:, 0]),
        "wk1": np.ascontiguousarray(wk[:, 1]),
        "bq": bqc, "bk": bkc,
        "wv": wv, "bv": bv_aug, "wfc": wfc,
    }


def _install_ntff_hook():
    """The container's antenv stub lacks axon_hooks; synthesize it so
    trace=True can capture NTFF profiles via libaxon_pjrt.so ctypes calls."""
    import contextlib
    import ctypes
    import types

    import antenv

    if hasattr(antenv, "axon_hooks"):
        return
    so_path = "/opt/axon/libaxon_pjrt.so"
    try:
        lib = ctypes.CDLL(so_path)
    except OSError:
        return
    if not hasattr(lib, "axon_start_nrt_profile"):
        return
    lib.axon_start_nrt_profile.argtypes = [ctypes.POINTER(ctypes.c_int64), ctypes.c_size_t]
    lib.axon_start_nrt_profile.restype = ctypes.c_int64
    lib.axon_stop_nrt_profile.argtypes = [ctypes.c_char_p]
    lib.axon_stop_nrt_profile.restype = ctypes.c_int64

    @contextlib.contextmanager
    def _hook(output_dir, device_ids):
        import jax

        jax.devices()
        if device_ids:
            ids = (ctypes.c_int64 * len(device_ids))(*device_ids)
            rc = lib.axon_start_nrt_profile(ids, len(device_ids))
        else:
            rc = lib.axon_start_nrt_profile(None, 0)
        if rc != 0:
            raise RuntimeError(f"axon_start_nrt_profile rc={rc}")
        try:
            yield
        finally:
            n = lib.axon_stop_nrt_profile(str(output_dir).encode())
            print(f"profile: {n} file(s) written to {output_dir}", file=sys.stderr)

    mod = types.ModuleType("antenv.axon_hooks")
    mod._hook = _hook
    mod.get_axon_ntff_profile_hook = lambda: _hook
    mod.set_axon_ntff_profile_hook = lambda h: setattr(mod, "_hook", h)
    sys.modules["antenv.axon_hooks"] = mod
    antenv.axon_hooks = mod


def kernel(_trace=False, **inputs):
    from concourse.bass_utils import run_bass_kernel_spmd

    if _trace:
        _install_ntff_hook()
    if "nc" not in _cached:
        _cached["nc"] = _build_program()
    nc = _cached["nc"]

    args = {k: np.asarray(v) for k, v in inputs.items()}
    eb_all = _prep_eb_all(args["relative_position_bias"])
    in_maps = [
        _prep_core_inputs(
            c,
            args["query"], args["key"], args["value"],
            eb_all,
            args["Wq"], args["bq"], args["Wk"], args["bk"],
            args["Wv"], args["bv"], args["Wfc"],
        )
        for c in range(NCORES)
    ]

    res = run_bass_kernel_spmd(nc, in_maps, core_ids=list(range(NCORES)), trace=_trace)
    _cached["last_result"] = res

    out = np.zeros((B, S, D), dtype=np.float32)
    cpb = NCORES // B
    for b in range(B):
        out[b] = sum(
            res.results[b * cpb + i]["out"].astype(np.float32) for i in range(cpb)
        )
        out[b] += args["bfc"].astype(np.float32)[None, :]
    return out
